# revision 1
# baseline (speedup 1.0000x reference)
"""MoE kernel for Trainium2 (8 NeuronCores), expert-parallel.

Strategy:
  - Host computes the (tiny) router: logits = x @ router_w in f64, softmax,
    top-2 expert indices + gate probs per token (verified to match
    jax.lax.top_k selection exactly on f32 ties-by-lower-index).
  - Tokens are gathered per routed expert on host (all-to-all dispatch done
    at input-sharding time). Core e receives its expert's tokens padded to
    capacity C (max expert load rounded to 128).
  - The shared expert is split along the FFN dim F: core e owns columns
    [e*512,(e+1)*512) of S_up and the matching rows of S_down, and computes
    a partial shared output for ALL tokens; the host sums the 8 partials
    (a sum over F-slices is exact in the FFN structure since only gelu is
    nonlinear and it is applied per-F-element before the down projection).
  - Device kernel per core, two phases with all weights SBUF-resident:
      phase S: partial shared FFN over all 8192 tokens (F-slice 512)
      phase R: own routed expert over C gathered tokens, gate fused into
               the PSUM eviction
    Matmuls in bf16 with f32 PSUM accumulation; exact-erf gelu on ScalarE.
    Phase S weights are tiny (4MB) so compute starts almost immediately;
    the 16MB routed weights stream in on the SWDGE queue behind it.
  - Host combines: y = x + sum_cores shared_partial + gather of gated
    routed outputs (each token's top-2 expert rows).
"""

import sys

if "/opt/trn_rl_repo" not in sys.path:
    sys.path.insert(0, "/opt/trn_rl_repo")

from contextlib import ExitStack

import ml_dtypes
import numpy as np

H, F, E, TOPK = 1024, 4096, 8, 2
N_CORES = 8
CHUNK = 256  # tokens per pipeline chunk (2 c-tiles of 128)
NOUT = 2  # h-output tiles of 512
FS = F // N_CORES  # shared-expert F-slice per core (512)
BF16 = ml_dtypes.bfloat16

_nc_cache = {}

# test-harness hooks (unused when graded): set TRACE=True to request an NTFF
# profile; the BassKernelResults of the last run lands in LAST_RESULT.
TRACE = False
LAST_RESULT = None


def _ffn_phase(nc, tile, dt, act, *, wu, wd, x_r, out_r, c_lo, c_hi, n_f,
               pools, g_sb=None, g_base=0, paced_dmas=None):
    """One dense FFN phase: out = [gate *] gelu(x @ Wup) @ Wdown.

    wu: list of k-tiles [128, n_f*128] (lhsT slices along H)
    wd: list of n_f tiles [128, H]
    x_r/out_r: DRAM APs [128, kt, tokens] / [128, tokens/128, H]
    """
    import concourse.mybir as mybir

    xpool, hpool, opool, pup, pdown = pools
    KT_H = H // 128
    GELU = getattr(mybir.ActivationFunctionType, act)

    n_chunks = -(-(c_hi - c_lo) // CHUNK)
    for ic, c0 in enumerate(range(c_lo, c_hi, CHUNK)):
        cc = min(CHUNK, c_hi - c0)
        nct = cc // 128
        x_sb = xpool.tile([128, KT_H, CHUNK], dt.bfloat16, tag="x")
        x_dma = nc.sync.dma_start(x_sb[:, :, :cc], x_r[:, :, c0 : c0 + cc])
        # one single-bank PSUM tile per (ci,ho) output slice: gives each
        # slice its own semaphore, so evictions start as soon as that
        # slice's accumulation stops and the next chunk's first down
        # matmuls wait only on their own slice's eviction.
        ps_d = [
            pdown.tile([128, 512], dt.float32, tag=f"pd{s}", name=f"pd{s}")
            for s in range(nct * NOUT)
        ]
        if paced_dmas:
            # pace bulk background DMAs (next phase's weights) across this
            # phase: emit a slice per chunk, gated on this chunk's x arrival
            # so they don't hog HBM bandwidth ahead of the compute stream.
            from concourse.bass import _add_dep_helper

            # skip the first chunks entirely: they prime the compute pipeline
            # and any HBM contention there stalls the PE directly
            skip = min(2, n_chunks - 1)
            span = n_chunks - skip
            lo = len(paced_dmas) * max(0, ic - skip) // span
            hi = len(paced_dmas) * max(0, ic - skip + 1) // span
            for fn in paced_dmas[lo:hi]:
                w_dma = fn()
                _add_dep_helper(
                    w_dma.ins, x_dma.ins, True, "paced background weight DMA"
                )
        # f-loop pipelined by one step: down(f) is emitted after up(f+1) so
        # the gelu -> LDWEIGHTS(hT) chain of step f hides under the up
        # matmuls of step f+1 instead of stalling the first down matmul.
        def emit_up(f):
            ps_u = pup.tile([128, cc], dt.float32, tag="pu")
            for kt in range(KT_H):
                nc.tensor.matmul(
                    ps_u[:],
                    wu[kt][:, f * 128 : (f + 1) * 128],
                    x_sb[:, kt, :cc],
                    start=(kt == 0),
                    stop=(kt == KT_H - 1),
                )
            hT = hpool.tile([128, cc], dt.bfloat16, tag="h")
            nc.scalar.activation(hT[:], ps_u[:], GELU)
            return hT

        def emit_down(f, hT):
            for ci in range(nct):
                for ho in range(NOUT):
                    nc.tensor.matmul(
                        ps_d[ci * NOUT + ho][:],
                        hT[:, ci * 128 : (ci + 1) * 128],
                        wd[f][:, ho * 512 : (ho + 1) * 512],
                        start=(f == 0),
                        stop=(f == n_f - 1),
                    )

        depth = 2 if n_f > 2 else 1
        hts = [emit_up(f) for f in range(min(depth, n_f))]
        for f in range(depth, n_f):
            hts.append(emit_up(f))
            emit_down(f - depth, hts[f - depth])
        for f in range(max(0, n_f - depth), n_f):
            emit_down(f, hts[f])

        for ci in range(nct):
            n = (c0 - c_lo) // 128 + ci
            o_sb = opool.tile([128, H], dt.float32, tag="o")
            for ho in range(NOUT):
                dst = o_sb[:, ho * 512 : (ho + 1) * 512]
                src = ps_d[ci * NOUT + ho][:]
                # split evictions across DVE and ACT (Copy/Identity share the
                # gelu PWP table set, so no table reload) — halves the
                # eviction latency the next chunk's down matmuls wait on
                if g_sb is not None:
                    g = g_sb[:, g_base + n : g_base + n + 1]
                    if ho % 2 == 0:
                        nc.vector.tensor_scalar_mul(dst, src, g)
                    else:
                        nc.scalar.activation(
                            dst, src, mybir.ActivationFunctionType.Copy, scale=g
                        )
                else:
                    if ho % 2 == 0:
                        nc.vector.tensor_copy(dst, src)
                    else:
                        nc.scalar.activation(
                            dst, src, mybir.ActivationFunctionType.Copy
                        )
            nc.sync.dma_start(out_r[:, n, :], o_sb[:])


def _build_nc(c_routed, t_total, act="Gelu"):
    import concourse.mybir as mybir
    import concourse.tile as tile
    from concourse import bacc

    dt = mybir.dt
    assert c_routed % 128 == 0 and t_total % CHUNK == 0
    KT_H = H // 128  # 8 k-tiles along H
    KT_F = F // 128  # 32 k-tiles along F (routed down-proj)
    NF_S = FS // 128  # 4 f-tiles in the shared slice

    # Bacc (not raw Bass): its compile pass splits sync waits down to the
    # TRN2 limit of 1 wait per instruction (walrus rejects multi-wait IR).
    nc = bacc.Bacc(None, target_bir_lowering=False)
    xT_r = nc.dram_tensor("xT_r", [H, c_routed], dt.bfloat16, kind="ExternalInput")
    xT_s = nc.dram_tensor("xT_s", [H, t_total], dt.bfloat16, kind="ExternalInput")
    gates = nc.dram_tensor(
        "gates", [128, c_routed // 128], dt.float32, kind="ExternalInput"
    )
    w_up = nc.dram_tensor("w_up", [H, F], dt.bfloat16, kind="ExternalInput")
    w_down = nc.dram_tensor("w_down", [F, H], dt.bfloat16, kind="ExternalInput")
    su_s = nc.dram_tensor("su_s", [H, FS], dt.bfloat16, kind="ExternalInput")
    sd_s = nc.dram_tensor("sd_s", [FS, H], dt.bfloat16, kind="ExternalInput")
    out_r = nc.dram_tensor("out_r", [c_routed, H], dt.float32, kind="ExternalOutput")
    out_s = nc.dram_tensor("out_s", [t_total, H], dt.float32, kind="ExternalOutput")

    xTr_t = xT_r.rearrange("(kt p) c -> p kt c", p=128)
    xTs_t = xT_s.rearrange("(kt p) c -> p kt c", p=128)
    outr_t = out_r.rearrange("(n p) h -> p n h", p=128)
    outs_t = out_s.rearrange("(n p) h -> p n h", p=128)

    with tile.TileContext(nc) as tc, ExitStack() as ctx:
        swpool = ctx.enter_context(tc.tile_pool(name="sweights", bufs=1))
        wpool = ctx.enter_context(tc.tile_pool(name="weights", bufs=1))
        xpool = ctx.enter_context(tc.tile_pool(name="x", bufs=3))
        hpool = ctx.enter_context(tc.tile_pool(name="h", bufs=6))
        cpool = ctx.enter_context(tc.tile_pool(name="const", bufs=1))
        opool = ctx.enter_context(tc.tile_pool(name="out", bufs=3))
        # 4 psd slices + 3 pup bufs = 7 of 8 PSUM banks; bufs=4 (all 8 banks)
        # crashes the device (NRT_EXEC_UNIT_UNRECOVERABLE) — do not fill PSUM.
        pup = ctx.enter_context(tc.tile_pool(name="pup", bufs=3, space="PSUM"))
        pdown = ctx.enter_context(tc.tile_pool(name="pdown", bufs=1, space="PSUM"))
        pools = (xpool, hpool, opool, pup, pdown)

        # shared-slice weights (small, on the HWDGE queue -> available fast);
        # one coalesced DMA each so SP-sequencer dispatch doesn't delay the
        # first x-chunk load behind a dozen small descriptors
        su_all = swpool.tile([128, KT_H, FS], dt.bfloat16, tag="su")
        nc.sync.dma_start(su_all[:], su_s.rearrange("(kt p) f -> p kt f", p=128)[:])
        su = [su_all[:, kt, :] for kt in range(KT_H)]
        sd_all = swpool.tile([128, NF_S, H], dt.bfloat16, tag="sd")
        nc.sync.dma_start(sd_all[:], sd_s.rearrange("(ft p) h -> p ft h", p=128)[:])
        sd = [sd_all[:, ft, :] for ft in range(NF_S)]

        # routed weights (16MB): tiles allocated now, DMAs deferred — they
        # are emitted paced across the shared phase (on the SWDGE queue) so
        # they don't steal HBM bandwidth from the shared phase's startup.
        wu, wd, w_dma_fns = [], [], []
        wu_t = w_up.rearrange("(kt p) f -> p kt f", p=128)
        for kt in range(KT_H):
            t = wpool.tile([128, F], dt.bfloat16, tag=f"wu{kt}")
            w_dma_fns.append(
                lambda t=t, kt=kt: nc.gpsimd.dma_start(t[:], wu_t[:, kt, :])
            )
            wu.append(t)
        wd_t = w_down.rearrange("(ft p) h -> p ft h", p=128)
        for ft in range(KT_F):
            t = wpool.tile([128, H], dt.bfloat16, tag=f"wd{ft}")
            w_dma_fns.append(
                lambda t=t, ft=ft: nc.gpsimd.dma_start(t[:], wd_t[:, ft, :])
            )
            wd.append(t)

        # phase S: partial shared FFN over all tokens, F-slice FS
        _ffn_phase(nc, tile, dt, act, wu=su, wd=sd, x_r=xTs_t, out_r=outs_t,
                   c_lo=0, c_hi=t_total, n_f=NF_S, pools=pools,
                   paced_dmas=w_dma_fns)

        g_sb = cpool.tile([128, c_routed // 128], dt.float32)
        nc.sync.dma_start(g_sb[:], gates[:])
        # phase R: routed expert over gathered tokens, gated eviction
        _ffn_phase(nc, tile, dt, act, wu=wu, wd=wd, x_r=xTr_t, out_r=outr_t,
                   c_lo=0, c_hi=c_routed, n_f=KT_F, pools=pools,
                   g_sb=g_sb)

    nc.finalize()
    return nc


def _get_nc(c_routed, t_total):
    key = (c_routed, t_total)
    if key not in _nc_cache:
        _nc_cache[key] = _build_nc(c_routed, t_total)
    return _nc_cache[key]


def _route(xf, router_w):
    """Host router in f64: top-2 indices (jax tie-break: lower index first)
    and their softmax probs."""
    logits = xf.astype(np.float64) @ router_w.astype(np.float64)
    m = logits.max(-1, keepdims=True)
    p = np.exp(logits - m)
    p /= p.sum(-1, keepdims=True)
    order = np.argsort(-p, axis=-1, kind="stable")
    top_idx = order[:, :TOPK]
    top_p = np.take_along_axis(p, top_idx, -1).astype(np.float32)
    return top_idx, top_p


def kernel(**inputs):
    x = np.ascontiguousarray(np.asarray(inputs["x"], np.float32))
    shared_up = np.asarray(inputs["shared_up"], np.float32)[0]
    shared_down = np.asarray(inputs["shared_down"], np.float32)[0]
    routed_up = np.asarray(inputs["routed_up"], np.float32)
    routed_down = np.asarray(inputs["routed_down"], np.float32)
    router_w = np.asarray(inputs["router_w"], np.float32)

    B, S, _ = x.shape
    T = B * S
    xf = x.reshape(T, H)

    top_idx, top_p = _route(xf, router_w)

    token_lists = [np.where((top_idx == e).any(-1))[0] for e in range(E)]
    c_cap = max(128, -(-max(len(l) for l in token_lists) // 128) * 128)

    # position of (token, slot) inside its expert's gathered buffer
    pos = np.zeros((T, TOPK), np.int64)
    gates_per_e = np.zeros((E, c_cap), np.float32)
    for e in range(E):
        lst = token_lists[e]
        for k in range(TOPK):
            sel = np.where(top_idx[:, k] == e)[0]
            p_in = np.searchsorted(lst, sel)
            pos[sel, k] = p_in
            gates_per_e[e, p_in] = top_p[sel, k]

    xf_bf = xf.astype(BF16)
    xTs = np.ascontiguousarray(xf_bf.T)  # [H, T], shared phase input
    su_bf = shared_up.astype(BF16)
    sd_bf = shared_down.astype(BF16)

    in_maps = []
    for e in range(E):
        lst = token_lists[e]
        xe = np.zeros((c_cap, H), BF16)
        xe[: len(lst)] = xf_bf[lst]
        in_maps.append(
            {
                "xT_r": np.ascontiguousarray(xe.T),
                "xT_s": xTs,
                "gates": np.ascontiguousarray(
                    gates_per_e[e].reshape(c_cap // 128, 128).T
                ),
                "w_up": routed_up[e].astype(BF16),
                "w_down": routed_down[e].astype(BF16),
                "su_s": np.ascontiguousarray(su_bf[:, e * FS : (e + 1) * FS]),
                "sd_s": np.ascontiguousarray(sd_bf[e * FS : (e + 1) * FS, :]),
            }
        )

    from concourse.bass_utils import run_bass_kernel_spmd

    nc = _get_nc(c_cap, T)
    res = run_bass_kernel_spmd(nc, in_maps, list(range(N_CORES)), trace=TRACE)
    global LAST_RESULT
    LAST_RESULT = res

    y = xf.copy()
    for e in range(E):
        y += res.results[e]["out_s"]
    y_routed = np.stack([res.results[e]["out_r"] for e in range(E)])  # gated rows
    for k in range(TOPK):
        y += y_routed[top_idx[:, k], pos[:, k]]
    return y.reshape(B, S, H)



# revision 2
# speedup vs baseline: 1.4878x; 1.4878x over previous
"""MoE kernel for Trainium2 (8 NeuronCores), expert-parallel.

Strategy:
  - Host computes the (tiny) router: logits = x @ router_w in f64, softmax,
    top-2 expert indices + gate probs per token (verified to match
    jax.lax.top_k selection exactly on f32 ties-by-lower-index).
  - Tokens are gathered per routed expert on host (all-to-all dispatch done
    at input-sharding time). Core e receives its expert's tokens padded to
    capacity C (max expert load rounded to 128).
  - The shared expert is split along the FFN dim F: core e owns columns
    [e*512,(e+1)*512) of S_up and the matching rows of S_down, and computes
    a partial shared output for ALL tokens in bf16; the host sums the 8
    partials (exact in the FFN structure since only gelu is nonlinear and
    it is applied per-F-element before the down projection).
  - The routed expert runs in fp8 e4m3 with DoubleRow perf mode (2 k-tiles
    of 128 contracted per PE pass at 2 rows/cycle): weights are host-scaled
    by 32 (escapes e4m3 denormals), the up-psum carries 32x which the gelu
    activation's scale=1/32 removes, and the down-psum's 32x is folded into
    the host-prepared gates (g/32) applied at PSUM eviction. The shared
    expert stays bf16 because its output is ungated (~0.4 rms vs the
    residual's 1.0) and dominates the fp8 quantization error budget;
    routed outputs are gated by ~0.1-0.3 so their fp8 error is benign
    (measured 8e-3 rel overall vs the 2e-2 budget).
  - Device kernel per core, two phases:
      phase S (bf16): partial shared FFN over all 8192 tokens (F-slice 512)
      phase R (fp8 DR): own routed expert over C gathered tokens, gate
               fused into the PSUM eviction
    f32 PSUM accumulation; exact-erf gelu on ScalarE. Phase S weights are
    tiny (4MB) so compute starts almost immediately; the 8MB fp8 routed
    weights stream in paced behind it on the SWDGE queue.
  - DoubleRow AP rules learned on hw: the moving (rhs) AP must be
    contiguous per partition (a strided 2xN slice of a wider tile wedges
    the device with NRT_EXEC_UNIT_UNRECOVERABLE); the stationary may be
    strided. Hence x chunks are allocated at their exact width and w_down
    is host-packed so every [128,2,512] rhs slice is one contiguous run.
  - Host combines: y = x + sum_cores shared_partial + gather of gated
    routed outputs (each token's top-2 expert rows), fp16 partials
    accumulated in f32.
"""

import sys

if "/opt/trn_rl_repo" not in sys.path:
    sys.path.insert(0, "/opt/trn_rl_repo")

from contextlib import ExitStack

import ml_dtypes
import numpy as np

H, F, E, TOPK = 1024, 4096, 8, 2
N_CORES = 8
CHUNK = 256  # tokens per pipeline chunk (2 c-tiles of 128)
NOUT = 2  # h-output tiles of 512
FS = F // N_CORES  # shared-expert F-slice per core (512)
WS = 32.0  # fp8 weight prescale (power of 2; exact)
BF16 = ml_dtypes.bfloat16
F8 = ml_dtypes.float8_e4m3
NFP = F // 256  # routed f-pairs (16)

_nc_cache = {}

# test-harness hooks (unused when graded): set TRACE=True to request an NTFF
# profile; the BassKernelResults of the last run lands in LAST_RESULT.
TRACE = False
LAST_RESULT = None


def _ffn_phase(nc, tile, dt, act, *, wu, wd, x_r, out_r, c_lo, c_hi, n_f,
               pools, paced_dmas=None):
    """One dense bf16 FFN phase: out = gelu(x @ Wup) @ Wdown.

    wu: list of k-tiles [128, n_f*128] (lhsT slices along H)
    wd: list of n_f tiles [128, H]
    x_r/out_r: DRAM APs [128, kt, tokens] / [128, tokens/128, H]
    """
    import concourse.mybir as mybir

    xpool, hpool, opool, pup, pdown = pools
    KT_H = H // 128
    GELU = getattr(mybir.ActivationFunctionType, act)

    n_chunks = -(-(c_hi - c_lo) // CHUNK)
    for ic, c0 in enumerate(range(c_lo, c_hi, CHUNK)):
        cc = min(CHUNK, c_hi - c0)
        nct = cc // 128
        x_sb = xpool.tile([128, KT_H, CHUNK], dt.bfloat16, tag="x")
        x_dma = nc.sync.dma_start(x_sb[:, :, :cc], x_r[:, :, c0 : c0 + cc])
        # one single-bank PSUM tile per (ci,ho) output slice: gives each
        # slice its own semaphore, so evictions start as soon as that
        # slice's accumulation stops and the next chunk's first down
        # matmuls wait only on their own slice's eviction.
        ps_d = [
            pdown.tile([128, 512], dt.float32, tag=f"pd{s}", name=f"pd{s}")
            for s in range(nct * NOUT)
        ]
        if paced_dmas:
            # pace bulk background DMAs (next phase's weights) across this
            # phase: emit a slice per chunk, gated on this chunk's x arrival
            # so they don't hog HBM bandwidth ahead of the compute stream.
            from concourse.bass import _add_dep_helper

            # skip the first chunks entirely: they prime the compute pipeline
            # and any HBM contention there stalls the PE directly
            skip = min(2, n_chunks - 1)
            span = n_chunks - skip
            lo = len(paced_dmas) * max(0, ic - skip) // span
            hi = len(paced_dmas) * max(0, ic - skip + 1) // span
            for fn in paced_dmas[lo:hi]:
                w_dma = fn()
                _add_dep_helper(
                    w_dma.ins, x_dma.ins, True, "paced background weight DMA"
                )
        # f-loop pipelined by one step: down(f) is emitted after up(f+1) so
        # the gelu -> LDWEIGHTS(hT) chain of step f hides under the up
        # matmuls of step f+1 instead of stalling the first down matmul.
        def emit_up(f):
            ps_u = pup.tile([128, cc], dt.float32, tag="pu")
            for kt in range(KT_H):
                nc.tensor.matmul(
                    ps_u[:],
                    wu[kt][:, f * 128 : (f + 1) * 128],
                    x_sb[:, kt, :cc],
                    start=(kt == 0),
                    stop=(kt == KT_H - 1),
                )
            hT = hpool.tile([128, cc], dt.bfloat16, tag="h")
            nc.scalar.activation(hT[:], ps_u[:], GELU)
            return hT

        def emit_down(f, hT):
            for ci in range(nct):
                for ho in range(NOUT):
                    nc.tensor.matmul(
                        ps_d[ci * NOUT + ho][:],
                        hT[:, ci * 128 : (ci + 1) * 128],
                        wd[f][:, ho * 512 : (ho + 1) * 512],
                        start=(f == 0),
                        stop=(f == n_f - 1),
                    )

        depth = 2 if n_f > 2 else 1
        hts = [emit_up(f) for f in range(min(depth, n_f))]
        for f in range(depth, n_f):
            hts.append(emit_up(f))
            emit_down(f - depth, hts[f - depth])
        for f in range(max(0, n_f - depth), n_f):
            emit_down(f, hts[f])

        for ci in range(nct):
            n = (c0 - c_lo) // 128 + ci
            o_sb = opool.tile([128, H], dt.float16, tag="o")
            for ho in range(NOUT):
                dst = o_sb[:, ho * 512 : (ho + 1) * 512]
                src = ps_d[ci * NOUT + ho][:]
                # split evictions across DVE and ACT (Copy/Identity share the
                # gelu PWP table set, so no table reload) — halves the
                # eviction latency the next chunk's down matmuls wait on
                if ho % 2 == 0:
                    nc.vector.tensor_copy(dst, src)
                else:
                    nc.scalar.activation(
                        dst, src, mybir.ActivationFunctionType.Copy
                    )
            nc.sync.dma_start(out_r[:, n, :], o_sb[:])


def _ffn_phase_dr(nc, tile, dt, act, *, wu_all, wd2, x_r, out_r, c_hi,
                  pools, g_sb):
    """Routed fp8 DoubleRow FFN phase: out = (g/WS) * (gelu((x@Wup)/WS) @ Wdown).

    wu_all: [128, KT_H, F] e4m3 tile (lhsT pairs along dim1)
    wd2:    [128, NFP, NOUT, 2, 512] e4m3 tile (contiguous DR rhs slices)
    x_r:    DRAM AP [128, kt, tokens] e4m3
    """
    import concourse.mybir as mybir

    xpool, hpool, opool, pup, pdown = pools
    KT_H = H // 128
    GELU = getattr(mybir.ActivationFunctionType, act)
    DRM = mybir.MatmulPerfMode.DoubleRow

    for c0 in range(0, c_hi, CHUNK):
        cc = min(CHUNK, c_hi - c0)
        nct = cc // 128
        # exact-width tile: DR moving APs must be contiguous per partition
        x_sb = xpool.tile([128, KT_H, cc], dt.float8e4, tag="xr")
        nc.sync.dma_start(x_sb[:], x_r[:, :, c0 : c0 + cc])
        ps_d = [
            pdown.tile([128, 512], dt.float32, tag=f"pd{s}", name=f"pd{s}")
            for s in range(nct * NOUT)
        ]

        def emit_up(f, h2, slot):
            ps_u = pup.tile([128, cc], dt.float32, tag="pu")
            for kp in range(KT_H // 2):
                nc.tensor.matmul(
                    ps_u[:],
                    wu_all[:, 2 * kp : 2 * kp + 2, f * 128 : (f + 1) * 128],
                    x_sb[:, 2 * kp : 2 * kp + 2, :],
                    start=(kp == 0),
                    stop=(kp == KT_H // 2 - 1),
                    perf_mode=DRM,
                )
            nc.scalar.activation(h2[:, slot, :], ps_u[:], GELU, scale=1.0 / WS)

        def emit_down(fp, h2):
            for ci in range(nct):
                for ho in range(NOUT):
                    nc.tensor.matmul(
                        ps_d[ci * NOUT + ho][:],
                        h2[:, :, ci * 128 : (ci + 1) * 128],
                        wd2[:, fp, ho, :, :],
                        start=(fp == 0),
                        stop=(fp == NFP - 1),
                        perf_mode=DRM,
                    )

        h2_prev = None
        for p in range(NFP):
            h2 = hpool.tile([128, 2, cc], dt.float8e4, tag="h2")
            emit_up(2 * p, h2, 0)
            emit_up(2 * p + 1, h2, 1)
            if h2_prev is not None:
                emit_down(p - 1, h2_prev)
            h2_prev = h2
        emit_down(NFP - 1, h2_prev)

        for ci in range(nct):
            n = c0 // 128 + ci
            o_sb = opool.tile([128, H], dt.float16, tag="o")
            g = g_sb[:, n : n + 1]
            for ho in range(NOUT):
                dst = o_sb[:, ho * 512 : (ho + 1) * 512]
                src = ps_d[ci * NOUT + ho][:]
                if ho % 2 == 0:
                    nc.vector.tensor_scalar_mul(dst, src, g)
                else:
                    nc.scalar.activation(
                        dst, src, mybir.ActivationFunctionType.Copy, scale=g
                    )
            nc.sync.dma_start(out_r[:, n, :], o_sb[:])


def _build_nc(c_routed, t_total, act="Gelu"):
    import concourse.mybir as mybir
    import concourse.tile as tile
    from concourse import bacc

    dt = mybir.dt
    assert c_routed % 128 == 0 and t_total % CHUNK == 0
    KT_H = H // 128  # 8 k-tiles along H
    NF_S = FS // 128  # 4 f-tiles in the shared slice

    # Bacc (not raw Bass): its compile pass splits sync waits down to the
    # TRN2 limit of 1 wait per instruction (walrus rejects multi-wait IR).
    nc = bacc.Bacc(None, target_bir_lowering=False)
    xT_r = nc.dram_tensor("xT_r", [H, c_routed], dt.float8e4, kind="ExternalInput")
    xT_s = nc.dram_tensor("xT_s", [H, t_total], dt.bfloat16, kind="ExternalInput")
    gates = nc.dram_tensor(
        "gates", [128, c_routed // 128], dt.float32, kind="ExternalInput"
    )
    w_up = nc.dram_tensor("w_up", [H, F], dt.float8e4, kind="ExternalInput")
    w_down = nc.dram_tensor(
        "w_down", [128, NFP * NOUT * 2 * 512], dt.float8e4, kind="ExternalInput"
    )
    su_s = nc.dram_tensor("su_s", [H, FS], dt.bfloat16, kind="ExternalInput")
    sd_s = nc.dram_tensor("sd_s", [FS, H], dt.bfloat16, kind="ExternalInput")
    out_r = nc.dram_tensor("out_r", [c_routed, H], dt.float16, kind="ExternalOutput")
    out_s = nc.dram_tensor("out_s", [t_total, H], dt.float16, kind="ExternalOutput")

    xTr_t = xT_r.rearrange("(kt p) c -> p kt c", p=128)
    xTs_t = xT_s.rearrange("(kt p) c -> p kt c", p=128)
    outr_t = out_r.rearrange("(n p) h -> p n h", p=128)
    outs_t = out_s.rearrange("(n p) h -> p n h", p=128)

    with tile.TileContext(nc) as tc, ExitStack() as ctx:
        swpool = ctx.enter_context(tc.tile_pool(name="sweights", bufs=1))
        wpool = ctx.enter_context(tc.tile_pool(name="weights", bufs=1))
        xpool = ctx.enter_context(tc.tile_pool(name="x", bufs=3))
        hpool = ctx.enter_context(tc.tile_pool(name="h", bufs=6))
        cpool = ctx.enter_context(tc.tile_pool(name="const", bufs=1))
        opool = ctx.enter_context(tc.tile_pool(name="out", bufs=3))
        # 4 psd slices + 3 pup bufs = 7 of 8 PSUM banks; bufs=4 (all 8 banks)
        # crashes the device (NRT_EXEC_UNIT_UNRECOVERABLE) — do not fill PSUM.
        pup = ctx.enter_context(tc.tile_pool(name="pup", bufs=3, space="PSUM"))
        pdown = ctx.enter_context(tc.tile_pool(name="pdown", bufs=1, space="PSUM"))
        pools = (xpool, hpool, opool, pup, pdown)

        # shared-slice weights (small, on the HWDGE queue -> available fast);
        # one coalesced DMA each so SP-sequencer dispatch doesn't delay the
        # first x-chunk load behind a dozen small descriptors
        su_all = swpool.tile([128, KT_H, FS], dt.bfloat16, tag="su")
        nc.sync.dma_start(su_all[:], su_s.rearrange("(kt p) f -> p kt f", p=128)[:])
        su = [su_all[:, kt, :] for kt in range(KT_H)]
        sd_all = swpool.tile([128, NF_S, H], dt.bfloat16, tag="sd")
        nc.sync.dma_start(sd_all[:], sd_s.rearrange("(ft p) h -> p ft h", p=128)[:])
        sd = [sd_all[:, ft, :] for ft in range(NF_S)]

        # routed fp8 weights (8MB): tiles allocated now, DMAs deferred — they
        # are emitted paced across the shared phase (on the SWDGE queue) so
        # they don't steal HBM bandwidth from the shared phase's startup.
        w_dma_fns = []
        wu_all = wpool.tile([128, KT_H, F], dt.float8e4, tag="wu")
        wu_t = w_up.rearrange("(kt p) f -> p kt f", p=128)
        for kt in range(KT_H):
            w_dma_fns.append(
                lambda kt=kt: nc.gpsimd.dma_start(wu_all[:, kt, :], wu_t[:, kt, :])
            )
        wd2 = wpool.tile([128, NFP, NOUT, 2, 512], dt.float8e4, tag="wd")
        wd_t = w_down.rearrange(
            "p (fp ho two j) -> p fp ho two j", fp=NFP, ho=NOUT, two=2
        )
        for fp in range(NFP):
            w_dma_fns.append(
                lambda fp=fp: nc.gpsimd.dma_start(wd2[:, fp], wd_t[:, fp])
            )

        # phase S: partial shared FFN over all tokens, F-slice FS (bf16)
        _ffn_phase(nc, tile, dt, act, wu=su, wd=sd, x_r=xTs_t, out_r=outs_t,
                   c_lo=0, c_hi=t_total, n_f=NF_S, pools=pools,
                   paced_dmas=w_dma_fns)

        g_sb = cpool.tile([128, c_routed // 128], dt.float32)
        nc.sync.dma_start(g_sb[:], gates[:])
        # phase R: routed expert over gathered tokens, fp8 DR, gated eviction
        _ffn_phase_dr(nc, tile, dt, act, wu_all=wu_all, wd2=wd2, x_r=xTr_t,
                      out_r=outr_t, c_hi=c_routed, pools=pools, g_sb=g_sb)

    nc.finalize()
    return nc


def _get_nc(c_routed, t_total):
    key = (c_routed, t_total)
    if key not in _nc_cache:
        _nc_cache[key] = _build_nc(c_routed, t_total)
    return _nc_cache[key]


def _route(xf, router_w):
    """Host router in f64: top-2 indices (jax tie-break: lower index first)
    and their softmax probs."""
    logits = xf.astype(np.float64) @ router_w.astype(np.float64)
    m = logits.max(-1, keepdims=True)
    p = np.exp(logits - m)
    p /= p.sum(-1, keepdims=True)
    order = np.argsort(-p, axis=-1, kind="stable")
    top_idx = order[:, :TOPK]
    top_p = np.take_along_axis(p, top_idx, -1).astype(np.float32)
    return top_idx, top_p


def _pack_wd(wd):
    """[F, H] f32 -> [128, NFP*NOUT*2*512] e4m3 in (p, fp, ho, pair, j) order
    so each DR rhs slice [128, 2, 512] is one contiguous run per partition."""
    a = (WS * wd).astype(F8).reshape(NFP, 2, 128, NOUT, 512)
    return np.ascontiguousarray(a.transpose(2, 0, 3, 1, 4)).reshape(128, -1)


def kernel(**inputs):
    x = np.ascontiguousarray(np.asarray(inputs["x"], np.float32))
    shared_up = np.asarray(inputs["shared_up"], np.float32)[0]
    shared_down = np.asarray(inputs["shared_down"], np.float32)[0]
    routed_up = np.asarray(inputs["routed_up"], np.float32)
    routed_down = np.asarray(inputs["routed_down"], np.float32)
    router_w = np.asarray(inputs["router_w"], np.float32)

    B, S, _ = x.shape
    T = B * S
    xf = x.reshape(T, H)

    top_idx, top_p = _route(xf, router_w)

    token_lists = [np.where((top_idx == e).any(-1))[0] for e in range(E)]
    c_cap = max(128, -(-max(len(l) for l in token_lists) // 128) * 128)

    # position of (token, slot) inside its expert's gathered buffer
    pos = np.zeros((T, TOPK), np.int64)
    gates_per_e = np.zeros((E, c_cap), np.float32)
    for e in range(E):
        lst = token_lists[e]
        for k in range(TOPK):
            sel = np.where(top_idx[:, k] == e)[0]
            p_in = np.searchsorted(lst, sel)
            pos[sel, k] = p_in
            gates_per_e[e, p_in] = top_p[sel, k]

    xf_bf = xf.astype(BF16)
    xTs = np.ascontiguousarray(xf_bf.T)  # [H, T], shared phase input
    xf_f8 = xf.astype(F8)
    su_bf = shared_up.astype(BF16)
    sd_bf = shared_down.astype(BF16)

    in_maps = []
    for e in range(E):
        lst = token_lists[e]
        xe = np.zeros((c_cap, H), F8)
        xe[: len(lst)] = xf_f8[lst]
        in_maps.append(
            {
                "xT_r": np.ascontiguousarray(xe.T),
                "xT_s": xTs,
                "gates": np.ascontiguousarray(
                    (gates_per_e[e] / WS).reshape(c_cap // 128, 128).T
                ),
                "w_up": (WS * routed_up[e]).astype(F8),
                "w_down": _pack_wd(routed_down[e]),
                "su_s": np.ascontiguousarray(su_bf[:, e * FS : (e + 1) * FS]),
                "sd_s": np.ascontiguousarray(sd_bf[e * FS : (e + 1) * FS, :]),
            }
        )

    from concourse.bass_utils import run_bass_kernel_spmd

    nc = _get_nc(c_cap, T)
    res = run_bass_kernel_spmd(nc, in_maps, list(range(N_CORES)), trace=TRACE)
    global LAST_RESULT
    LAST_RESULT = res

    y = xf.copy()
    for e in range(E):
        y += res.results[e]["out_s"].astype(np.float32)
    y_routed = np.stack(
        [res.results[e]["out_r"].astype(np.float32) for e in range(E)]
    )  # gated rows
    for k in range(TOPK):
        y += y_routed[top_idx[:, k], pos[:, k]]
    return y.reshape(B, S, H)


# revision 6
# speedup vs baseline: 1.6000x; 1.0755x over previous
"""MoE kernel for Trainium2 (8 NeuronCores), expert-parallel.

Strategy:
  - Host computes the (tiny) router: logits = x @ router_w in f64, softmax,
    top-2 expert indices + gate probs per token (verified to match
    jax.lax.top_k selection exactly on f32 ties-by-lower-index).
  - Tokens are gathered per routed expert on host (all-to-all dispatch done
    at input-sharding time). Core e receives its expert's tokens padded to
    capacity C (max expert load rounded to 128).
  - The shared expert is split along the FFN dim F: core e owns columns
    [e*512,(e+1)*512) of S_up and the matching rows of S_down, and computes
    a partial shared output for ALL tokens; the host sums the 8 partials
    (exact in the FFN structure since only gelu is nonlinear and it is
    applied per-F-element before the down projection).
  - fp8 e4m3 with DoubleRow perf mode (2 k-tiles of 128 contracted per PE
    pass at 2 rows/cycle = 157 TF/s vs bf16's 78.6) carries most of the
    compute: weights are host-scaled by 32 (escapes e4m3 denormals), the
    up-psum's 32x is removed by the gelu activation's scale=1/32, and the
    down-psum's 32x is folded into the eviction scale (host-prepared g/32
    gates for routed; a 1/32 constant for shared).
  - Error budget (2e-2 gate): fp8 on the ungated shared expert costs
    ~5.6e-4 rel-err^2 spread over its 4 quantization points, vs only
    ~0.65e-4 for ALL of the gated routed experts (gates ~0.1-0.3 shrink
    their error 4x). So the routed phase is fully fp8 and the shared
    phase converts only the first N_FP8_CHUNKS token-chunks to fp8
    (err^2 ~ alpha * 5.6e-4), staying bf16 for the rest.
  - Device kernel per core, three phases:
      phase S16 (bf16): shared F-slice over chunks [N_FP8_CHUNKS, 32)
      phase S8 (fp8 DR): shared F-slice over chunks [0, N_FP8_CHUNKS)
      phase R (fp8 DR): own routed expert over C gathered tokens, gate
               fused into the PSUM eviction
    f32 PSUM accumulation; exact-erf gelu on ScalarE. Phase S16 weights are
    tiny (2MB) so compute starts almost immediately; the 8MB fp8 routed
    weights stream in paced behind it on the SWDGE queue.
  - DoubleRow AP rules learned on hw: the moving (rhs) AP must be
    contiguous per partition (a strided 2xN slice of a wider tile wedges
    the device with NRT_EXEC_UNIT_UNRECOVERABLE); the stationary may be
    strided. Hence x chunks are allocated at their exact width and down
    weights are host-packed so every [128,2,512] rhs slice is contiguous.
  - Host combines: y = x + sum_cores shared_partial + gather of gated
    routed outputs (each token's top-2 expert rows), fp16 partials
    accumulated in f32.
"""

import sys

if "/opt/trn_rl_repo" not in sys.path:
    sys.path.insert(0, "/opt/trn_rl_repo")

from contextlib import ExitStack

import ml_dtypes
import numpy as np

H, F, E, TOPK = 1024, 4096, 8, 2
N_CORES = 8
CHUNK = 256  # tokens per pipeline chunk (2 c-tiles of 128)
NOUT = 2  # h-output tiles of 512
FS = F // N_CORES  # shared-expert F-slice per core (512)
WS = 32.0  # fp8 weight prescale (power of 2; exact)
BF16 = ml_dtypes.bfloat16
F8 = ml_dtypes.float8_e4m3
N_FP8_CHUNKS = 13  # shared-phase chunks (of 32) run in fp8 (alpha ~ 0.41)

_nc_cache = {}

# test-harness hooks (unused when graded): set TRACE=True to request an NTFF
# profile; the BassKernelResults of the last run lands in LAST_RESULT.
TRACE = False
LAST_RESULT = None


def _ffn_phase(nc, tile, dt, act, *, wu, wd, x_r, out_r, c_lo, c_hi, n_f,
               pools, paced_dmas=None, out_n0=0):
    """One dense bf16 FFN phase: out = gelu(x @ Wup) @ Wdown.

    wu: list of k-tiles [128, n_f*128] (lhsT slices along H)
    wd: list of n_f tiles [128, H]
    x_r/out_r: DRAM APs [128, kt, tokens] / [128, tokens/128, H]
    c_lo/c_hi index x_r columns; out rows start at out_n0.
    """
    import concourse.mybir as mybir

    xpool, hpool, opool, pup, pdown = pools
    KT_H = H // 128
    GELU = getattr(mybir.ActivationFunctionType, act)

    n_chunks = -(-(c_hi - c_lo) // CHUNK)
    for ic, c0 in enumerate(range(c_lo, c_hi, CHUNK)):
        cc = min(CHUNK, c_hi - c0)
        nct = cc // 128
        x_sb = xpool.tile([128, KT_H, CHUNK], dt.bfloat16, tag="x")
        x_dma = nc.sync.dma_start(x_sb[:, :, :cc], x_r[:, :, c0 : c0 + cc])
        # one single-bank PSUM tile per (ci,ho) output slice: gives each
        # slice its own semaphore, so evictions start as soon as that
        # slice's accumulation stops and the next chunk's first down
        # matmuls wait only on their own slice's eviction.
        ps_d = [
            pdown.tile([128, 512], dt.float32, tag=f"pd{s}", name=f"pd{s}")
            for s in range(nct * NOUT)
        ]
        if paced_dmas:
            # pace bulk background DMAs (next phase's weights) across this
            # phase: emit a slice per chunk, gated on this chunk's x arrival
            # so they don't hog HBM bandwidth ahead of the compute stream.
            from concourse.bass import _add_dep_helper

            # skip the first chunks entirely: they prime the compute pipeline
            # and any HBM contention there stalls the PE directly
            skip = min(2, n_chunks - 1)
            span = n_chunks - skip
            lo = len(paced_dmas) * max(0, ic - skip) // span
            hi = len(paced_dmas) * max(0, ic - skip + 1) // span
            for fn in paced_dmas[lo:hi]:
                w_dma = fn()
                _add_dep_helper(
                    w_dma.ins, x_dma.ins, True, "paced background weight DMA"
                )
        # f-loop pipelined by one step: down(f) is emitted after up(f+1) so
        # the gelu -> LDWEIGHTS(hT) chain of step f hides under the up
        # matmuls of step f+1 instead of stalling the first down matmul.
        def emit_up(f):
            ps_u = pup.tile([128, cc], dt.float32, tag="pu")
            for kt in range(KT_H):
                nc.tensor.matmul(
                    ps_u[:],
                    wu[kt][:, f * 128 : (f + 1) * 128],
                    x_sb[:, kt, :cc],
                    start=(kt == 0),
                    stop=(kt == KT_H - 1),
                )
            hT = hpool.tile([128, cc], dt.bfloat16, tag="h")
            nc.scalar.activation(hT[:], ps_u[:], GELU)
            return hT

        def emit_down(f, hT):
            for ci in range(nct):
                for ho in range(NOUT):
                    nc.tensor.matmul(
                        ps_d[ci * NOUT + ho][:],
                        hT[:, ci * 128 : (ci + 1) * 128],
                        wd[f][:, ho * 512 : (ho + 1) * 512],
                        start=(f == 0),
                        stop=(f == n_f - 1),
                    )

        depth = 2 if n_f > 2 else 1
        hts = [emit_up(f) for f in range(min(depth, n_f))]
        for f in range(depth, n_f):
            hts.append(emit_up(f))
            emit_down(f - depth, hts[f - depth])
        for f in range(max(0, n_f - depth), n_f):
            emit_down(f, hts[f])

        for ci in range(nct):
            n = out_n0 + (c0 - c_lo) // 128 + ci
            o_sb = opool.tile([128, H], dt.float16, tag="o")
            for ho in range(NOUT):
                dst = o_sb[:, ho * 512 : (ho + 1) * 512]
                src = ps_d[ci * NOUT + ho][:]
                # split evictions across DVE and ACT (Copy/Identity share the
                # gelu PWP table set, so no table reload) — halves the
                # eviction latency the next chunk's down matmuls wait on
                if ho % 2 == 0:
                    nc.vector.tensor_copy(dst, src)
                else:
                    nc.scalar.activation(
                        dst, src, mybir.ActivationFunctionType.Copy
                    )
            nc.sync.dma_start(out_r[:, n, :], o_sb[:])


def _ffn_phase_dr(nc, tile, dt, act, *, wu_all, wd2, x_r, out_r, c_hi,
                  n_f, pools, g_sb, g_col, out_n0=0):
    """fp8 DoubleRow FFN phase: out = scale * (gelu((x@Wup)/WS) @ Wdown)
    where scale = g/WS per token (g_col=None walks g_sb columns) or a
    constant 1/WS broadcast (g_col fixes one g_sb column).

    wu_all: [128, KT_H, n_f*128] e4m3 tile (lhsT pairs along dim1)
    wd2:    [128, n_f//2, NOUT, 2, 512] e4m3 tile (contiguous DR rhs slices)
    x_r:    DRAM AP [128, kt, tokens] e4m3
    """
    import concourse.mybir as mybir

    xpool, hpool, opool, pup, pdown = pools
    KT_H = H // 128
    GELU = getattr(mybir.ActivationFunctionType, act)
    DRM = mybir.MatmulPerfMode.DoubleRow
    nfp = n_f // 2

    for c0 in range(0, c_hi, CHUNK):
        cc = min(CHUNK, c_hi - c0)
        nct = cc // 128
        # exact-width tile: DR moving APs must be contiguous per partition
        x_sb = xpool.tile([128, KT_H, cc], dt.float8e4, tag="xr")
        nc.sync.dma_start(x_sb[:], x_r[:, :, c0 : c0 + cc])
        ps_d = [
            pdown.tile([128, 512], dt.float32, tag=f"pd{s}", name=f"pd{s}")
            for s in range(nct * NOUT)
        ]

        def emit_up(f, h2, slot):
            ps_u = pup.tile([128, cc], dt.float32, tag="pu")
            for kp in range(KT_H // 2):
                nc.tensor.matmul(
                    ps_u[:],
                    wu_all[:, 2 * kp : 2 * kp + 2, f * 128 : (f + 1) * 128],
                    x_sb[:, 2 * kp : 2 * kp + 2, :],
                    start=(kp == 0),
                    stop=(kp == KT_H // 2 - 1),
                    perf_mode=DRM,
                )
            nc.scalar.activation(h2[:, slot, :], ps_u[:], GELU, scale=1.0 / WS)

        def emit_down(fp, h2):
            for ci in range(nct):
                for ho in range(NOUT):
                    nc.tensor.matmul(
                        ps_d[ci * NOUT + ho][:],
                        h2[:, :, ci * 128 : (ci + 1) * 128],
                        wd2[:, fp, ho, :, :],
                        start=(fp == 0),
                        stop=(fp == nfp - 1),
                        perf_mode=DRM,
                    )

        h2_prev = None
        for p in range(nfp):
            h2 = hpool.tile([128, 2, cc], dt.float8e4, tag="h2")
            emit_up(2 * p, h2, 0)
            emit_up(2 * p + 1, h2, 1)
            if h2_prev is not None:
                emit_down(p - 1, h2_prev)
            h2_prev = h2
        emit_down(nfp - 1, h2_prev)

        for ci in range(nct):
            n = c0 // 128 + ci
            o_sb = opool.tile([128, H], dt.float16, tag="o")
            col = n if g_col is None else g_col
            g = g_sb[:, col : col + 1]
            for ho in range(NOUT):
                dst = o_sb[:, ho * 512 : (ho + 1) * 512]
                src = ps_d[ci * NOUT + ho][:]
                if ho % 2 == 0:
                    nc.vector.tensor_scalar_mul(dst, src, g)
                else:
                    nc.scalar.activation(
                        dst, src, mybir.ActivationFunctionType.Copy, scale=g
                    )
            nc.sync.dma_start(out_r[:, out_n0 + n, :], o_sb[:])


def _build_nc(c_routed, t_total, act="Gelu", n8=N_FP8_CHUNKS):
    import concourse.mybir as mybir
    import concourse.tile as tile
    from concourse import bacc

    dt = mybir.dt
    assert c_routed % 128 == 0 and t_total % CHUNK == 0
    KT_H = H // 128  # 8 k-tiles along H
    NF_S = FS // 128  # 4 f-tiles in the shared slice
    NFP = F // 256  # 16 routed f-pairs
    t8 = n8 * CHUNK  # tokens handled by the fp8 shared sub-phase

    # Bacc (not raw Bass): its compile pass splits sync waits down to the
    # TRN2 limit of 1 wait per instruction (walrus rejects multi-wait IR).
    nc = bacc.Bacc(None, target_bir_lowering=False)
    xT_r = nc.dram_tensor("xT_r", [H, c_routed], dt.float8e4, kind="ExternalInput")
    xT_s = nc.dram_tensor(
        "xT_s", [H, t_total - t8], dt.bfloat16, kind="ExternalInput"
    )
    xT_s8 = nc.dram_tensor("xT_s8", [H, max(t8, CHUNK)], dt.float8e4,
                           kind="ExternalInput")
    gates = nc.dram_tensor(
        "gates", [128, c_routed // 128 + 1], dt.float32, kind="ExternalInput"
    )
    w_up = nc.dram_tensor("w_up", [H, F], dt.float8e4, kind="ExternalInput")
    w_down = nc.dram_tensor(
        "w_down", [128, NFP * NOUT * 2 * 512], dt.float8e4, kind="ExternalInput"
    )
    su_s = nc.dram_tensor("su_s", [H, FS], dt.bfloat16, kind="ExternalInput")
    sd_s = nc.dram_tensor("sd_s", [FS, H], dt.bfloat16, kind="ExternalInput")
    su_s8 = nc.dram_tensor("su_s8", [H, FS], dt.float8e4, kind="ExternalInput")
    sd_s8 = nc.dram_tensor(
        "sd_s8", [128, (NF_S // 2) * NOUT * 2 * 512], dt.float8e4,
        kind="ExternalInput",
    )
    out_r = nc.dram_tensor("out_r", [c_routed, H], dt.float16, kind="ExternalOutput")
    out_s = nc.dram_tensor("out_s", [t_total, H], dt.float16, kind="ExternalOutput")

    xTr_t = xT_r.rearrange("(kt p) c -> p kt c", p=128)
    xTs_t = xT_s.rearrange("(kt p) c -> p kt c", p=128)
    xTs8_t = xT_s8.rearrange("(kt p) c -> p kt c", p=128)
    outr_t = out_r.rearrange("(n p) h -> p n h", p=128)
    outs_t = out_s.rearrange("(n p) h -> p n h", p=128)

    with tile.TileContext(nc) as tc, ExitStack() as ctx:
        swpool = ctx.enter_context(tc.tile_pool(name="sweights", bufs=1))
        wpool = ctx.enter_context(tc.tile_pool(name="weights", bufs=1))
        xpool = ctx.enter_context(tc.tile_pool(name="x", bufs=3))
        hpool = ctx.enter_context(tc.tile_pool(name="h", bufs=6))
        cpool = ctx.enter_context(tc.tile_pool(name="const", bufs=1))
        opool = ctx.enter_context(tc.tile_pool(name="out", bufs=3))
        # 4 psd slices + 3 pup bufs = 7 of 8 PSUM banks; bufs=4 (all 8 banks)
        # crashes the device (NRT_EXEC_UNIT_UNRECOVERABLE) — do not fill PSUM.
        pup = ctx.enter_context(tc.tile_pool(name="pup", bufs=3, space="PSUM"))
        pdown = ctx.enter_context(tc.tile_pool(name="pdown", bufs=1, space="PSUM"))
        pools = (xpool, hpool, opool, pup, pdown)

        # shared-slice weights: bf16 set on the HWDGE queue (needed first),
        # fp8 set + gates on the SWDGE queue ahead of the paced routed bulk.
        # One coalesced DMA each so SP-sequencer dispatch doesn't delay the
        # first x-chunk load behind a dozen small descriptors.
        su_all = swpool.tile([128, KT_H, FS], dt.bfloat16, tag="su")
        nc.sync.dma_start(su_all[:], su_s.rearrange("(kt p) f -> p kt f", p=128)[:])
        su = [su_all[:, kt, :] for kt in range(KT_H)]
        sd_all = swpool.tile([128, NF_S, H], dt.bfloat16, tag="sd")
        nc.sync.dma_start(sd_all[:], sd_s.rearrange("(ft p) h -> p ft h", p=128)[:])
        sd = [sd_all[:, ft, :] for ft in range(NF_S)]

        su8 = swpool.tile([128, KT_H, FS], dt.float8e4, tag="su8")
        nc.gpsimd.dma_start(su8[:], su_s8.rearrange("(kt p) f -> p kt f", p=128)[:])
        sd8 = swpool.tile([128, NF_S // 2, NOUT, 2, 512], dt.float8e4, tag="sd8")
        nc.gpsimd.dma_start(
            sd8[:],
            sd_s8.rearrange(
                "p (fp ho two j) -> p fp ho two j", fp=NF_S // 2, ho=NOUT, two=2
            )[:],
        )
        # gates + the 1/WS shared eviction constant live in one tile; the
        # constant sits in the extra last column.
        g_sb = cpool.tile([128, c_routed // 128 + 1], dt.float32)
        nc.gpsimd.dma_start(g_sb[:], gates[:])

        # routed fp8 weights (8MB): tiles allocated now, DMAs deferred — they
        # are emitted paced across the bf16 shared phase (on the SWDGE queue)
        # so they don't steal HBM bandwidth from the shared phase's startup.
        w_dma_fns = []
        wu_all = wpool.tile([128, KT_H, F], dt.float8e4, tag="wu")
        wu_t = w_up.rearrange("(kt p) f -> p kt f", p=128)
        for kt in range(KT_H):
            w_dma_fns.append(
                lambda kt=kt: nc.gpsimd.dma_start(wu_all[:, kt, :], wu_t[:, kt, :])
            )
        wd2 = wpool.tile([128, NFP, NOUT, 2, 512], dt.float8e4, tag="wd")
        wd_t = w_down.rearrange(
            "p (fp ho two j) -> p fp ho two j", fp=NFP, ho=NOUT, two=2
        )
        for fp in range(NFP):
            w_dma_fns.append(
                lambda fp=fp: nc.gpsimd.dma_start(wd2[:, fp], wd_t[:, fp])
            )

        # phase S16: bf16 shared FFN over chunks [n8, 32); its x tensor
        # holds only tokens [t8, T) so x columns are relative
        _ffn_phase(nc, tile, dt, act, wu=su, wd=sd, x_r=xTs_t, out_r=outs_t,
                   c_lo=0, c_hi=t_total - t8, n_f=NF_S, pools=pools,
                   paced_dmas=w_dma_fns, out_n0=t8 // 128)

        # phase S8: fp8 DR shared FFN over chunks [0, n8)
        if n8 > 0:
            _ffn_phase_dr(nc, tile, dt, act, wu_all=su8, wd2=sd8, x_r=xTs8_t,
                          out_r=outs_t, c_hi=t8, n_f=NF_S, pools=pools,
                          g_sb=g_sb, g_col=c_routed // 128, out_n0=0)

        # phase R: routed expert over gathered tokens, fp8 DR, gated eviction
        _ffn_phase_dr(nc, tile, dt, act, wu_all=wu_all, wd2=wd2, x_r=xTr_t,
                      out_r=outr_t, c_hi=c_routed, n_f=F // 128, pools=pools,
                      g_sb=g_sb, g_col=None,
                      out_n0=0)

    nc.finalize()
    return nc


def _get_nc(c_routed, t_total):
    key = (c_routed, t_total)
    if key not in _nc_cache:
        _nc_cache[key] = _build_nc(c_routed, t_total)
    return _nc_cache[key]


def _route(xf, router_w):
    """Host router in f64: top-2 indices (jax tie-break: lower index first)
    and their softmax probs."""
    logits = xf.astype(np.float64) @ router_w.astype(np.float64)
    m = logits.max(-1, keepdims=True)
    p = np.exp(logits - m)
    p /= p.sum(-1, keepdims=True)
    order = np.argsort(-p, axis=-1, kind="stable")
    top_idx = order[:, :TOPK]
    top_p = np.take_along_axis(p, top_idx, -1).astype(np.float32)
    return top_idx, top_p


def _pack_wd(wd):
    """[Fd, H] f32 -> [128, (Fd/256)*NOUT*2*512] e4m3 in (p, fp, ho, pair, j)
    order so each DR rhs slice [128, 2, 512] is one contiguous run."""
    nfp = wd.shape[0] // 256
    a = (WS * wd).astype(F8).reshape(nfp, 2, 128, NOUT, 512)
    return np.ascontiguousarray(a.transpose(2, 0, 3, 1, 4)).reshape(128, -1)


def kernel(**inputs):
    x = np.ascontiguousarray(np.asarray(inputs["x"], np.float32))
    shared_up = np.asarray(inputs["shared_up"], np.float32)[0]
    shared_down = np.asarray(inputs["shared_down"], np.float32)[0]
    routed_up = np.asarray(inputs["routed_up"], np.float32)
    routed_down = np.asarray(inputs["routed_down"], np.float32)
    router_w = np.asarray(inputs["router_w"], np.float32)

    B, S, _ = x.shape
    T = B * S
    t8 = N_FP8_CHUNKS * CHUNK
    xf = x.reshape(T, H)

    top_idx, top_p = _route(xf, router_w)

    token_lists = [np.where((top_idx == e).any(-1))[0] for e in range(E)]
    c_cap = max(128, -(-max(len(l) for l in token_lists) // 128) * 128)

    # position of (token, slot) inside its expert's gathered buffer
    pos = np.zeros((T, TOPK), np.int64)
    gates_per_e = np.zeros((E, c_cap), np.float32)
    for e in range(E):
        lst = token_lists[e]
        for k in range(TOPK):
            sel = np.where(top_idx[:, k] == e)[0]
            p_in = np.searchsorted(lst, sel)
            pos[sel, k] = p_in
            gates_per_e[e, p_in] = top_p[sel, k]

    xf_bf = xf.astype(BF16)
    xf_f8 = xf.astype(F8)
    xTs = np.ascontiguousarray(xf_bf[t8:].T)  # [H, T-t8], bf16 shared input
    xTs8 = np.ascontiguousarray(xf_f8[: max(t8, CHUNK)].T)  # fp8 shared input
    su_bf = shared_up.astype(BF16)
    sd_bf = shared_down.astype(BF16)

    in_maps = []
    for e in range(E):
        lst = token_lists[e]
        xe = np.zeros((c_cap, H), F8)
        xe[: len(lst)] = xf_f8[lst]
        gcols = np.empty((c_cap // 128 + 1, 128), np.float32)
        gcols[: c_cap // 128] = (gates_per_e[e] / WS).reshape(-1, 128)
        gcols[c_cap // 128] = 1.0 / WS  # shared-phase eviction constant
        su_sl = shared_up[:, e * FS : (e + 1) * FS]
        sd_sl = shared_down[e * FS : (e + 1) * FS, :]
        in_maps.append(
            {
                "xT_r": np.ascontiguousarray(xe.T),
                "xT_s": xTs,
                "xT_s8": xTs8,
                "gates": np.ascontiguousarray(gcols.T),
                "w_up": (WS * routed_up[e]).astype(F8),
                "w_down": _pack_wd(routed_down[e]),
                "su_s": np.ascontiguousarray(su_bf[:, e * FS : (e + 1) * FS]),
                "sd_s": np.ascontiguousarray(sd_bf[e * FS : (e + 1) * FS, :]),
                "su_s8": np.ascontiguousarray((WS * su_sl).astype(F8)),
                "sd_s8": _pack_wd(sd_sl),
            }
        )

    from concourse.bass_utils import run_bass_kernel_spmd

    nc = _get_nc(c_cap, T)
    res = run_bass_kernel_spmd(nc, in_maps, list(range(N_CORES)), trace=TRACE)
    global LAST_RESULT
    LAST_RESULT = res

    y = xf.copy()
    for e in range(E):
        y += res.results[e]["out_s"].astype(np.float32)
    y_routed = np.stack(
        [res.results[e]["out_r"].astype(np.float32) for e in range(E)]
    )  # gated rows
    for k in range(TOPK):
        y += y_routed[top_idx[:, k], pos[:, k]]
    return y.reshape(B, S, H)


# revision 9
# speedup vs baseline: 1.6767x; 1.0479x over previous
"""MoE kernel for Trainium2 (8 NeuronCores), expert-parallel.

Strategy:
  - Host computes the (tiny) router: logits = x @ router_w in f64, softmax,
    top-2 expert indices + gate probs per token (verified to match
    jax.lax.top_k selection exactly on f32 ties-by-lower-index).
  - Tokens are gathered per routed expert on host (all-to-all dispatch done
    at input-sharding time). Core e receives its expert's tokens padded to
    capacity C (max expert load rounded to 128).
  - The shared expert is split along the FFN dim F: core e owns columns
    [e*512,(e+1)*512) of S_up and the matching rows of S_down, and computes
    a partial shared output for ALL tokens; the host sums the 8 partials
    (exact in the FFN structure since only gelu is nonlinear and it is
    applied per-F-element before the down projection).
  - fp8 e4m3 with DoubleRow perf mode (2 k-tiles of 128 contracted per PE
    pass at 2 rows/cycle = 157 TF/s vs bf16's 78.6) carries most of the
    compute: weights are host-scaled by 32 (escapes e4m3 denormals), the
    up-psum's 32x is removed by the gelu activation's scale=1/32, and the
    down-psum's 32x is folded into the eviction scale (host-prepared g/32
    gates for routed; a 1/32 constant for shared).
  - Error budget (2e-2 gate): fp8 on the ungated shared expert costs
    ~5.6e-4 rel-err^2 spread over its 4 quantization points, vs only
    ~0.65e-4 for ALL of the gated routed experts (gates ~0.1-0.3 shrink
    their error 4x). So the routed phase is fully fp8 and the shared
    phase converts only the first N_FP8_CHUNKS token-chunks to fp8
    (err^2 ~ alpha * 5.6e-4), staying bf16 for the rest.
  - Device kernel per core, three phases:
      phase S16 (bf16): shared F-slice over chunks [N_FP8_CHUNKS, 32)
      phase S8 (fp8 DR): shared F-slice over chunks [0, N_FP8_CHUNKS)
      phase R (fp8 DR): own routed expert over C gathered tokens, gate
               fused into the PSUM eviction
    f32 PSUM accumulation; exact-erf gelu on ScalarE. Phase S16 weights are
    tiny (2MB) so compute starts almost immediately; the 8MB fp8 routed
    weights stream in paced behind it on the SWDGE queue.
  - DoubleRow AP rules learned on hw: the moving (rhs) AP must be
    contiguous per partition (a strided 2xN slice of a wider tile wedges
    the device with NRT_EXEC_UNIT_UNRECOVERABLE); the stationary may be
    strided. Hence x chunks are allocated at their exact width and down
    weights are host-packed so every [128,2,512] rhs slice is contiguous.
  - Host combines: y = x + sum_cores shared_partial + gather of gated
    routed outputs (each token's top-2 expert rows), fp16 partials
    accumulated in f32.
"""

import sys

if "/opt/trn_rl_repo" not in sys.path:
    sys.path.insert(0, "/opt/trn_rl_repo")

from contextlib import ExitStack

import ml_dtypes
import numpy as np

H, F, E, TOPK = 1024, 4096, 8, 2
N_CORES = 8
CHUNK = 256  # tokens per pipeline chunk (2 c-tiles of 128)
NOUT = 2  # h-output tiles of 512
FS = F // N_CORES  # shared-expert F-slice per core (512)
WS = 32.0  # fp8 weight prescale (power of 2; exact)
BF16 = ml_dtypes.bfloat16
F8 = ml_dtypes.float8_e4m3
N_FP8_CHUNKS = 15  # shared-phase chunks (of 32) run in fp8 (alpha ~ 0.47)

_nc_cache = {}

# test-harness hooks (unused when graded): set TRACE=True to request an NTFF
# profile; the BassKernelResults of the last run lands in LAST_RESULT.
TRACE = False
LAST_RESULT = None


def _ffn_phase(nc, tile, dt, act, *, wu, wd, x_r, out_r, c_lo, c_hi, n_f,
               pools, paced_dmas=None, out_n0=0):
    """One dense bf16 FFN phase: out = gelu(x @ Wup) @ Wdown.

    wu: list of k-tiles [128, n_f*128] (lhsT slices along H)
    wd: list of n_f tiles [128, H]
    x_r/out_r: DRAM APs [128, kt, tokens] / [128, tokens/128, H]
    c_lo/c_hi index x_r columns; out rows start at out_n0.
    """
    import concourse.mybir as mybir

    xpool, hpool, opool, pup, pdown = pools
    KT_H = H // 128
    GELU = getattr(mybir.ActivationFunctionType, act)

    n_chunks = -(-(c_hi - c_lo) // CHUNK)
    for ic, c0 in enumerate(range(c_lo, c_hi, CHUNK)):
        cc = min(CHUNK, c_hi - c0)
        nct = cc // 128
        x_sb = xpool.tile([128, KT_H, CHUNK], dt.bfloat16, tag="x")
        x_dma = nc.sync.dma_start(x_sb[:, :, :cc], x_r[:, :, c0 : c0 + cc])
        # one single-bank PSUM tile per (ci,ho) output slice: gives each
        # slice its own semaphore, so evictions start as soon as that
        # slice's accumulation stops and the next chunk's first down
        # matmuls wait only on their own slice's eviction.
        ps_d = [
            pdown.tile([128, 512], dt.float32, tag=f"pd{s}", name=f"pd{s}")
            for s in range(nct * NOUT)
        ]
        if paced_dmas:
            # pace bulk background DMAs (next phase's weights) across this
            # phase: emit a slice per chunk, gated on this chunk's x arrival
            # so they don't hog HBM bandwidth ahead of the compute stream.
            from concourse.bass import _add_dep_helper

            # skip the first chunks entirely: they prime the compute pipeline
            # and any HBM contention there stalls the PE directly
            skip = min(2, n_chunks - 1)
            span = n_chunks - skip
            lo = len(paced_dmas) * max(0, ic - skip) // span
            hi = len(paced_dmas) * max(0, ic - skip + 1) // span
            for fn in paced_dmas[lo:hi]:
                w_dma = fn()
                _add_dep_helper(
                    w_dma.ins, x_dma.ins, True, "paced background weight DMA"
                )
        # f-loop pipelined by one step: down(f) is emitted after up(f+1) so
        # the gelu -> LDWEIGHTS(hT) chain of step f hides under the up
        # matmuls of step f+1 instead of stalling the first down matmul.
        def emit_up(f):
            ps_u = pup.tile([128, cc], dt.float32, tag="pu")
            for kt in range(KT_H):
                nc.tensor.matmul(
                    ps_u[:],
                    wu[kt][:, f * 128 : (f + 1) * 128],
                    x_sb[:, kt, :cc],
                    start=(kt == 0),
                    stop=(kt == KT_H - 1),
                )
            hT = hpool.tile([128, cc], dt.bfloat16, tag="h")
            nc.scalar.activation(hT[:], ps_u[:], GELU)
            return hT

        def emit_down(f, hT):
            for ci in range(nct):
                for ho in range(NOUT):
                    nc.tensor.matmul(
                        ps_d[ci * NOUT + ho][:],
                        hT[:, ci * 128 : (ci + 1) * 128],
                        wd[f][:, ho * 512 : (ho + 1) * 512],
                        start=(f == 0),
                        stop=(f == n_f - 1),
                    )

        depth = 2 if n_f > 2 else 1
        hts = [emit_up(f) for f in range(min(depth, n_f))]
        for f in range(depth, n_f):
            hts.append(emit_up(f))
            emit_down(f - depth, hts[f - depth])
        for f in range(max(0, n_f - depth), n_f):
            emit_down(f, hts[f])

        for ci in range(nct):
            n = out_n0 + (c0 - c_lo) // 128 + ci
            o_sb = opool.tile([128, H], dt.float16, tag="o")
            for ho in range(NOUT):
                dst = o_sb[:, ho * 512 : (ho + 1) * 512]
                src = ps_d[ci * NOUT + ho][:]
                # split evictions across DVE and ACT (Copy/Identity share the
                # gelu PWP table set, so no table reload) — halves the
                # eviction latency the next chunk's down matmuls wait on
                if ho % 2 == 0:
                    nc.vector.tensor_copy(dst, src)
                else:
                    nc.scalar.activation(
                        dst, src, mybir.ActivationFunctionType.Copy
                    )
            nc.sync.dma_start(out_r[:, n, :], o_sb[:])


def _ffn_phase_dr(nc, tile, dt, act, *, wu_all, wd2, x_r, out_r, c_hi,
                  n_f, pools, g_sb, g_col, out_n0=0):
    """fp8 DoubleRow FFN phase: out = scale * (gelu((x@Wup)/WS) @ Wdown)
    where scale = g/WS per token (g_col=None walks g_sb columns) or a
    constant 1/WS broadcast (g_col fixes one g_sb column).

    wu_all: [128, KT_H, n_f*128] e4m3 tile (lhsT pairs along dim1)
    wd2:    [128, n_f//2, NOUT, 2, 512] e4m3 tile (contiguous DR rhs slices)
    x_r:    DRAM AP [128, kt, tokens] e4m3
    """
    import concourse.mybir as mybir

    xpool, hpool, opool, pup, pdown = pools
    KT_H = H // 128
    GELU = getattr(mybir.ActivationFunctionType, act)
    DRM = mybir.MatmulPerfMode.DoubleRow
    nfp = n_f // 2

    for c0 in range(0, c_hi, CHUNK):
        cc = min(CHUNK, c_hi - c0)
        nct = cc // 128
        # exact-width tile: DR moving APs must be contiguous per partition
        x_sb = xpool.tile([128, KT_H, cc], dt.float8e4, tag="xr")
        nc.sync.dma_start(x_sb[:], x_r[:, :, c0 : c0 + cc])
        ps_d = [
            pdown.tile([128, 512], dt.float32, tag=f"pd{s}", name=f"pd{s}")
            for s in range(nct * NOUT)
        ]

        def emit_up(f, h2, slot):
            ps_u = pup.tile([128, cc], dt.float32, tag="pu")
            for kp in range(KT_H // 2):
                nc.tensor.matmul(
                    ps_u[:],
                    wu_all[:, 2 * kp : 2 * kp + 2, f * 128 : (f + 1) * 128],
                    x_sb[:, 2 * kp : 2 * kp + 2, :],
                    start=(kp == 0),
                    stop=(kp == KT_H // 2 - 1),
                    perf_mode=DRM,
                )
            nc.scalar.activation(h2[:, slot, :], ps_u[:], GELU, scale=1.0 / WS)

        def emit_down(fp, h2):
            for ci in range(nct):
                for ho in range(NOUT):
                    nc.tensor.matmul(
                        ps_d[ci * NOUT + ho][:],
                        h2[:, :, ci * 128 : (ci + 1) * 128],
                        wd2[:, fp, ho, :, :],
                        start=(fp == 0),
                        stop=(fp == nfp - 1),
                        perf_mode=DRM,
                    )

        h2_prev = None
        for p in range(nfp):
            h2 = hpool.tile([128, 2, cc], dt.float8e4, tag="h2")
            emit_up(2 * p, h2, 0)
            emit_up(2 * p + 1, h2, 1)
            if h2_prev is not None:
                emit_down(p - 1, h2_prev)
            h2_prev = h2
        emit_down(nfp - 1, h2_prev)

        for ci in range(nct):
            n = c0 // 128 + ci
            o_sb = opool.tile([128, H], dt.float16, tag="o")
            col = n if g_col is None else g_col
            g = g_sb[:, col : col + 1]
            for ho in range(NOUT):
                dst = o_sb[:, ho * 512 : (ho + 1) * 512]
                src = ps_d[ci * NOUT + ho][:]
                # all evictions on DVE: ScalarE must stay free for the next
                # chunk's gelus — they gate the first down matmuls' h2
                # LDWEIGHTS (measured 650ns/chunk PE bubble when ACT-queued)
                nc.vector.tensor_scalar_mul(dst, src, g)
            nc.sync.dma_start(out_r[:, out_n0 + n, :], o_sb[:])


def _build_nc(c_routed, t_total, act="Gelu", n8=N_FP8_CHUNKS):
    import concourse.mybir as mybir
    import concourse.tile as tile
    from concourse import bacc

    dt = mybir.dt
    assert c_routed % 128 == 0 and t_total % CHUNK == 0
    KT_H = H // 128  # 8 k-tiles along H
    NF_S = FS // 128  # 4 f-tiles in the shared slice
    NFP = F // 256  # 16 routed f-pairs
    t8 = n8 * CHUNK  # tokens handled by the fp8 shared sub-phase

    # Bacc (not raw Bass): its compile pass splits sync waits down to the
    # TRN2 limit of 1 wait per instruction (walrus rejects multi-wait IR).
    nc = bacc.Bacc(None, target_bir_lowering=False)
    xT_r = nc.dram_tensor("xT_r", [H, c_routed], dt.float8e4, kind="ExternalInput")
    xT_s = nc.dram_tensor(
        "xT_s", [H, t_total - t8], dt.bfloat16, kind="ExternalInput"
    )
    xT_s8 = nc.dram_tensor("xT_s8", [H, max(t8, CHUNK)], dt.float8e4,
                           kind="ExternalInput")
    gates = nc.dram_tensor(
        "gates", [128, c_routed // 128 + 1], dt.float32, kind="ExternalInput"
    )
    w_up = nc.dram_tensor("w_up", [H, F], dt.float8e4, kind="ExternalInput")
    w_down = nc.dram_tensor(
        "w_down", [128, NFP * NOUT * 2 * 512], dt.float8e4, kind="ExternalInput"
    )
    su_s = nc.dram_tensor("su_s", [H, FS], dt.bfloat16, kind="ExternalInput")
    sd_s = nc.dram_tensor("sd_s", [FS, H], dt.bfloat16, kind="ExternalInput")
    su_s8 = nc.dram_tensor("su_s8", [H, FS], dt.float8e4, kind="ExternalInput")
    sd_s8 = nc.dram_tensor(
        "sd_s8", [128, (NF_S // 2) * NOUT * 2 * 512], dt.float8e4,
        kind="ExternalInput",
    )
    out_r = nc.dram_tensor("out_r", [c_routed, H], dt.float16, kind="ExternalOutput")
    out_s = nc.dram_tensor("out_s", [t_total, H], dt.float16, kind="ExternalOutput")

    xTr_t = xT_r.rearrange("(kt p) c -> p kt c", p=128)
    xTs_t = xT_s.rearrange("(kt p) c -> p kt c", p=128)
    xTs8_t = xT_s8.rearrange("(kt p) c -> p kt c", p=128)
    outr_t = out_r.rearrange("(n p) h -> p n h", p=128)
    outs_t = out_s.rearrange("(n p) h -> p n h", p=128)

    with tile.TileContext(nc) as tc, ExitStack() as ctx:
        swpool = ctx.enter_context(tc.tile_pool(name="sweights", bufs=1))
        wpool = ctx.enter_context(tc.tile_pool(name="weights", bufs=1))
        xpool = ctx.enter_context(tc.tile_pool(name="x", bufs=3))
        hpool = ctx.enter_context(tc.tile_pool(name="h", bufs=6))
        cpool = ctx.enter_context(tc.tile_pool(name="const", bufs=1))
        opool = ctx.enter_context(tc.tile_pool(name="out", bufs=3))
        # 4 psd slices + 3 pup bufs = 7 of 8 PSUM banks; bufs=4 (all 8 banks)
        # crashes the device (NRT_EXEC_UNIT_UNRECOVERABLE) — do not fill PSUM.
        pup = ctx.enter_context(tc.tile_pool(name="pup", bufs=3, space="PSUM"))
        pdown = ctx.enter_context(tc.tile_pool(name="pdown", bufs=1, space="PSUM"))
        pools = (xpool, hpool, opool, pup, pdown)

        # shared-slice weights: bf16 set on the HWDGE queue (needed first),
        # fp8 set + gates on the SWDGE queue ahead of the paced routed bulk.
        # One coalesced DMA each so SP-sequencer dispatch doesn't delay the
        # first x-chunk load behind a dozen small descriptors.
        su_all = swpool.tile([128, KT_H, FS], dt.bfloat16, tag="su")
        nc.gpsimd.dma_start(su_all[:], su_s.rearrange("(kt p) f -> p kt f", p=128)[:])
        su = [su_all[:, kt, :] for kt in range(KT_H)]
        sd_all = swpool.tile([128, NF_S, H], dt.bfloat16, tag="sd")
        nc.gpsimd.dma_start(sd_all[:], sd_s.rearrange("(ft p) h -> p ft h", p=128)[:])
        sd = [sd_all[:, ft, :] for ft in range(NF_S)]

        su8 = swpool.tile([128, KT_H, FS], dt.float8e4, tag="su8")
        nc.gpsimd.dma_start(su8[:], su_s8.rearrange("(kt p) f -> p kt f", p=128)[:])
        sd8 = swpool.tile([128, NF_S // 2, NOUT, 2, 512], dt.float8e4, tag="sd8")
        nc.gpsimd.dma_start(
            sd8[:],
            sd_s8.rearrange(
                "p (fp ho two j) -> p fp ho two j", fp=NF_S // 2, ho=NOUT, two=2
            )[:],
        )
        # gates + the 1/WS shared eviction constant live in one tile; the
        # constant sits in the extra last column.
        g_sb = cpool.tile([128, c_routed // 128 + 1], dt.float32)
        nc.gpsimd.dma_start(g_sb[:], gates[:])

        # routed fp8 weights (8MB): tiles allocated now, DMAs deferred — they
        # are emitted paced across the bf16 shared phase (on the SWDGE queue)
        # so they don't steal HBM bandwidth from the shared phase's startup.
        w_dma_fns = []
        wu_all = wpool.tile([128, KT_H, F], dt.float8e4, tag="wu")
        wu_t = w_up.rearrange("(kt p) f -> p kt f", p=128)
        for kt in range(KT_H):
            w_dma_fns.append(
                lambda kt=kt: nc.gpsimd.dma_start(wu_all[:, kt, :], wu_t[:, kt, :])
            )
        wd2 = wpool.tile([128, NFP, NOUT, 2, 512], dt.float8e4, tag="wd")
        wd_t = w_down.rearrange(
            "p (fp ho two j) -> p fp ho two j", fp=NFP, ho=NOUT, two=2
        )
        for fp in range(NFP):
            w_dma_fns.append(
                lambda fp=fp: nc.gpsimd.dma_start(wd2[:, fp], wd_t[:, fp])
            )

        # phase S16: bf16 shared FFN over chunks [n8, 32); its x tensor
        # holds only tokens [t8, T) so x columns are relative
        _ffn_phase(nc, tile, dt, act, wu=su, wd=sd, x_r=xTs_t, out_r=outs_t,
                   c_lo=0, c_hi=t_total - t8, n_f=NF_S, pools=pools,
                   paced_dmas=w_dma_fns, out_n0=t8 // 128)

        # phase S8: fp8 DR shared FFN over chunks [0, n8)
        if n8 > 0:
            _ffn_phase_dr(nc, tile, dt, act, wu_all=su8, wd2=sd8, x_r=xTs8_t,
                          out_r=outs_t, c_hi=t8, n_f=NF_S, pools=pools,
                          g_sb=g_sb, g_col=c_routed // 128, out_n0=0)

        # phase R: routed expert over gathered tokens, fp8 DR, gated eviction
        _ffn_phase_dr(nc, tile, dt, act, wu_all=wu_all, wd2=wd2, x_r=xTr_t,
                      out_r=outr_t, c_hi=c_routed, n_f=F // 128, pools=pools,
                      g_sb=g_sb, g_col=None,
                      out_n0=0)

    nc.finalize()
    return nc


def _get_nc(c_routed, t_total):
    key = (c_routed, t_total)
    if key not in _nc_cache:
        _nc_cache[key] = _build_nc(c_routed, t_total)
    return _nc_cache[key]


def _route(xf, router_w):
    """Host router in f64: top-2 indices (jax tie-break: lower index first)
    and their softmax probs."""
    logits = xf.astype(np.float64) @ router_w.astype(np.float64)
    m = logits.max(-1, keepdims=True)
    p = np.exp(logits - m)
    p /= p.sum(-1, keepdims=True)
    order = np.argsort(-p, axis=-1, kind="stable")
    top_idx = order[:, :TOPK]
    top_p = np.take_along_axis(p, top_idx, -1).astype(np.float32)
    return top_idx, top_p


def _pack_wd(wd):
    """[Fd, H] f32 -> [128, (Fd/256)*NOUT*2*512] e4m3 in (p, fp, ho, pair, j)
    order so each DR rhs slice [128, 2, 512] is one contiguous run."""
    nfp = wd.shape[0] // 256
    a = (WS * wd).astype(F8).reshape(nfp, 2, 128, NOUT, 512)
    return np.ascontiguousarray(a.transpose(2, 0, 3, 1, 4)).reshape(128, -1)


def kernel(**inputs):
    x = np.ascontiguousarray(np.asarray(inputs["x"], np.float32))
    shared_up = np.asarray(inputs["shared_up"], np.float32)[0]
    shared_down = np.asarray(inputs["shared_down"], np.float32)[0]
    routed_up = np.asarray(inputs["routed_up"], np.float32)
    routed_down = np.asarray(inputs["routed_down"], np.float32)
    router_w = np.asarray(inputs["router_w"], np.float32)

    B, S, _ = x.shape
    T = B * S
    t8 = N_FP8_CHUNKS * CHUNK
    xf = x.reshape(T, H)

    top_idx, top_p = _route(xf, router_w)

    token_lists = [np.where((top_idx == e).any(-1))[0] for e in range(E)]
    c_cap = max(128, -(-max(len(l) for l in token_lists) // 128) * 128)

    # position of (token, slot) inside its expert's gathered buffer
    pos = np.zeros((T, TOPK), np.int64)
    gates_per_e = np.zeros((E, c_cap), np.float32)
    for e in range(E):
        lst = token_lists[e]
        for k in range(TOPK):
            sel = np.where(top_idx[:, k] == e)[0]
            p_in = np.searchsorted(lst, sel)
            pos[sel, k] = p_in
            gates_per_e[e, p_in] = top_p[sel, k]

    xf_bf = xf.astype(BF16)
    xf_f8 = xf.astype(F8)
    xTs = np.ascontiguousarray(xf_bf[t8:].T)  # [H, T-t8], bf16 shared input
    xTs8 = np.ascontiguousarray(xf_f8[: max(t8, CHUNK)].T)  # fp8 shared input
    su_bf = shared_up.astype(BF16)
    sd_bf = shared_down.astype(BF16)

    in_maps = []
    for e in range(E):
        lst = token_lists[e]
        xe = np.zeros((c_cap, H), F8)
        xe[: len(lst)] = xf_f8[lst]
        gcols = np.empty((c_cap // 128 + 1, 128), np.float32)
        gcols[: c_cap // 128] = (gates_per_e[e] / WS).reshape(-1, 128)
        gcols[c_cap // 128] = 1.0 / WS  # shared-phase eviction constant
        su_sl = shared_up[:, e * FS : (e + 1) * FS]
        sd_sl = shared_down[e * FS : (e + 1) * FS, :]
        in_maps.append(
            {
                "xT_r": np.ascontiguousarray(xe.T),
                "xT_s": xTs,
                "xT_s8": xTs8,
                "gates": np.ascontiguousarray(gcols.T),
                "w_up": (WS * routed_up[e]).astype(F8),
                "w_down": _pack_wd(routed_down[e]),
                "su_s": np.ascontiguousarray(su_bf[:, e * FS : (e + 1) * FS]),
                "sd_s": np.ascontiguousarray(sd_bf[e * FS : (e + 1) * FS, :]),
                "su_s8": np.ascontiguousarray((WS * su_sl).astype(F8)),
                "sd_s8": _pack_wd(sd_sl),
            }
        )

    from concourse.bass_utils import run_bass_kernel_spmd

    nc = _get_nc(c_cap, T)
    res = run_bass_kernel_spmd(nc, in_maps, list(range(N_CORES)), trace=TRACE)
    global LAST_RESULT
    LAST_RESULT = res

    y = xf.copy()
    for e in range(E):
        y += res.results[e]["out_s"].astype(np.float32)
    y_routed = np.stack(
        [res.results[e]["out_r"].astype(np.float32) for e in range(E)]
    )  # gated rows
    for k in range(TOPK):
        y += y_routed[top_idx[:, k], pos[:, k]]
    return y.reshape(B, S, H)
